# revision 2
# baseline (speedup 1.0000x reference)
"""Trainium2 Bass kernel for nn_Basic_MPNN (gnn_message_passing).

Math (per batch b):
  m1 = node @ W1 + b1                  [N, MID]   (receiver side, axis 2)
  m2 = node @ W2 + b2                  [N, MID]   (sender side, axis 1)
  me = edge @ We + be                  [N, N, MID]
  mg = graph @ Wg + bg                 [MID]
  msgs[j,i,:] = m1[i] + m2[j] + me[j,i] + mg
  M[i,:] = max_j where(adj[j,i], msgs[j,i,:], -1e6)
  out = relu(node @ Wo1 + bo1 + M @ Wo2 + bo2)

Sharding: 8 cores = (4 batches) x (2 receiver halves of 256).

Per-core algorithm (balanced across PE / DMA / ACT / DVE):
  Receivers are i-paired: SBUF partition p of an edge tile holds i = 2p+e
  (parity e in {0,1} replaces the receiver-halves "ib" axis).  This makes
  both sides of the casting edge-load DMA >= 512B contiguous, so the DMA
  runs at full modeled bandwidth (1456 ns per 8-sender group).  All
  i-indexed host tensors (adj, noderT) are parity-permuted on the host and
  the output is inverse-permuted in numpy.

  Per 8-sender group: DMA the [128p, 8j*2e*128d] f16 tile.  A tunable
  subset of groups is transposed by the DMA xbar (dma_start_transpose,
  one instr per group -> tf [d, i] tiles directly in SBUF); the remaining
  groups use PE transposes + PSUM evictions split between ACT and DVE
  (DVE tensor_copy of f16 runs in 2x mode).  Stationary-We fp16 matmuls
  produce meT slots [mid, j*128+i] in PSUM; per j a rank-2 matmul
  accumulates adj01[j,i]*m2[j,mid] + (1-adj01[j,i])*(-60000), applying
  mask and sender term exactly.

  Reduction over senders (DVE can read only ONE PSUM operand/instr):
  ACT evicts slots 4-7 to SBUF f32; DVE folds max(PSUM slots 0-3,
  evicted) -> f16; a second DVE tensor_tensor (f16, 2x mode) folds into a
  per-parity running max acc[e] [mid, 4*128].  Finalize folds acc to
  [mid, 128], adds cT = (m1 + mg + biases)^T, clamps, and applies the
  output matmuls + relu.

Rank-2 row-group placement: all rank-2 matmuls of sender-group g share PE
row-group k = g // 16 (two adjacent row-grouped matmuls with *different*
tile_position inside an open PSUM accumulation group crash the HW --
verified experimentally; same tile_position back-to-back is fine, and a
full-K matmul between them is fine). The build asserts the final PE
schedule has no unsafe adjacency.
"""

import os
import sys

for _p in (
    "/root/.axon_site",
    "/root/.axon_site/_ro/trn_rl_repo",
    "/root/.axon_site/_ro/pypackages",
    "/opt/trn_rl_repo",
    "/opt/pypackages",
):
    if os.path.isdir(_p) and _p not in sys.path:
        sys.path.append(_p)

import numpy as np  # noqa: E402

import concourse.bass as bass  # noqa: E402
import concourse.tile as tile  # noqa: E402
from concourse import bacc, masks, mybir  # noqa: E402
from concourse.bass_utils import run_bass_kernel_spmd  # noqa: E402

F32 = mybir.dt.float32
F16 = mybir.dt.float16
I32 = mybir.dt.int32

B, N, D, MID, OUT = 4, 512, 128, 128, 128
NCORES = 8
IH = N // 2  # receivers per core
JG = 8       # senders per j-group
NG = N // JG  # 64 j-groups
MASK_NEG = -60000.0  # < any valid msg value; fp16-representable
BIG_NUMBER = 1.0e6

# groups transposed by the DMA xbar instead of the PE (tunable balance)
N_DMA_GROUPS = 26
DMA_GROUPS = frozenset(int((k + 0.5) * NG / N_DMA_GROUPS) for k in range(N_DMA_GROUPS))
# of the 4 PSUM->SBUF tf copies per PE-group, every 3rd goes to DVE
DVE_COPY_MOD = 3


def _k_of_group(g):
    # row-group for sender-group g; constant across 16-group spans so
    # adjacent rank-2 matmuls share tile_position almost everywhere
    return g // 16


def _u_of_j(j):
    return j % 128


def _build_program(repeat=1):
    nc = bacc.Bacc(
        "TRN2", target_bir_lowering=False, debug=False, num_devices=NCORES
    )

    edge = nc.dram_tensor("edge", [N, IH, D], F32, kind="ExternalInput").ap()
    nodeT_d = nc.dram_tensor("nodeT", [D, N], F32, kind="ExternalInput").ap()
    noderT_d = nc.dram_tensor("noderT", [D, IH], F32, kind="ExternalInput").ap()
    graph = nc.dram_tensor("graph", [1, D], F32, kind="ExternalInput").ap()
    adj = nc.dram_tensor("adj", [N, IH], I32, kind="ExternalInput").ap()
    wpack_d = nc.dram_tensor("wpack", [D, 5 * MID], F32, kind="ExternalInput").ap()
    bpack_d = nc.dram_tensor("bpack", [1, 6 * MID], F32, kind="ExternalInput").ap()
    we_d = nc.dram_tensor("We", [D, MID], F32, kind="ExternalInput").ap()
    out_d = nc.dram_tensor("out", [IH, OUT], F32, kind="ExternalOutput").ap()

    with (
        tile.TileContext(nc) as tc,
        tc.tile_pool(name="persist", bufs=1) as pp,
        tc.tile_pool(name="setup_sb", bufs=1) as ssb,
        tc.tile_pool(name="accum", bufs=1) as accp,
        tc.tile_pool(name="edge", bufs=6) as ep,
        tc.tile_pool(name="tf", bufs=8) as tfp,
        tc.tile_pool(name="tfb", bufs=3) as tfbp,
        tc.tile_pool(name="ev", bufs=3) as evp,
        tc.tile_pool(name="t16", bufs=3) as t16p,
        tc.tile_pool(name="ps8", bufs=3, space="PSUM") as ps8p,
        tc.tile_pool(name="psT", bufs=2, space="PSUM") as psTp,
    ):
        if True:
            # ---------------- adjacency in rank-2 rhs layout ----------------
            # adjn[u, k, i] = adj[128k+u, i]  (i already parity-permuted on host)
            adjn = ssb.tile([128, 4 * IH], I32)
            nc.sync.dma_start(
                adjn[:], adj.rearrange("(k u) i -> u k i", k=4)
            )
            a01_32 = ssb.tile([128, 4 * IH], F32)
            nc.vector.tensor_copy(a01_32[:], adjn[:])
            a01 = ssb.tile([128, 4 * IH], F16)
            nc.vector.tensor_copy(a01[:], a01_32[:])
            inv01 = ssb.tile([128, 4 * IH], F16)
            nc.vector.tensor_scalar(
                inv01[:], a01_32[:], -1.0, 1.0,
                op0=mybir.AluOpType.mult, op1=mybir.AluOpType.add,
            )
            # adjr2[32k+0, u*256 + e*128 + c] = adj01[j, e*128+c]
            adjr2 = pp.tile([128, 128 * IH], F16)
            for k in range(4):
                nc.sync.dma_start(
                    adjr2[32 * k:32 * k + 1, :], a01[:, k * IH:(k + 1) * IH]
                )
                nc.scalar.dma_start(
                    adjr2[32 * k + 1:32 * k + 2, :],
                    inv01[:, k * IH:(k + 1) * IH],
                )
            # ---------------- constants & weights ----------------
            ident16 = pp.tile([128, 128], F16)
            masks.make_identity(nc, ident16[:])
            ones32 = pp.tile([1, 256], F32)
            nc.vector.memset(ones32[:], 1.0)

            # node features first: they gate the m2 -> m2r2 chain
            nodeT = pp.tile([D, N], F32)
            nc.sync.dma_start(nodeT[:], nodeT_d[:, :])
            noderT = pp.tile([D, IH], F32)
            nc.scalar.dma_start(noderT[:], noderT_d[:, :])
            wpack = pp.tile([D, 5 * MID], F32)
            nc.sync.dma_start(wpack[:], wpack_d[:, :])
            bpack = pp.tile([1, 6 * MID], F32)
            nc.scalar.dma_start(bpack[:], bpack_d[:, :])
            wsb = {
                w: wpack[:, i * MID:(i + 1) * MID]
                for i, w in enumerate(("W2", "W1", "Wg", "Wo1", "Wo2"))
            }
            bsb = {
                b: bpack[:, i * MID:(i + 1) * MID]
                for i, b in enumerate(("b1", "b2", "be", "bg", "bo1", "bo2"))
            }
            we16 = pp.tile([D, MID], F16)
            nc.gpsimd.dma_start(we16[:], we_d[:, :])  # cast f32->f16

            # ---------------- m2 in rank-2 lhsT layout ----------------
            # m2r2[32k+0, u*128+mid] = m2[j, mid] (f16), j = 128k + u;
            # m2r2[32k+1, ...] = MASK_NEG
            m2r2 = pp.tile([128, 128 * MID], F16)
            neg_sb = ssb.tile([128, 512], F16)
            nc.vector.memset(neg_sb[:], MASK_NEG)
            m2f16 = ssb.tile([128, 4 * MID], F16)
            # nodeT columns j = 128k + u
            for k in range(4):
                ps_m2 = psTp.tile([128, MID], F32, tag="pT")
                nc.tensor.matmul(
                    ps_m2[:],
                    lhsT=nodeT[:, k * 128:(k + 1) * 128],
                    rhs=wsb["W2"], start=True, stop=False,
                )
                nc.tensor.matmul(
                    ps_m2[:], lhsT=ones32[:, 0:128], rhs=bsb["b2"],
                    start=False, stop=True,
                )
                nc.scalar.copy(m2f16[:, k * MID:(k + 1) * MID], ps_m2[:])
            for k in range(4):
                nc.sync.dma_start(
                    m2r2[32 * k:32 * k + 1, :],
                    m2f16[:, k * MID:(k + 1) * MID],
                )
                nc.scalar.dma_start(
                    m2r2[32 * k + 1:32 * k + 2, :], neg_sb[0:32, :]
                )

            # r = mg + b1 + be + bg ; bso = bo1 + bo2
            gT = ssb.tile([D, 1], F32)
            nc.sync.dma_start(gT[:], graph[0:1, :])
            ps_mg = psTp.tile([1, MID], F32, tag="pT")
            nc.tensor.matmul(ps_mg[:], lhsT=gT[:], rhs=wsb["Wg"], start=True, stop=True)
            r_sb = pp.tile([1, MID], F32)
            nc.scalar.copy(r_sb[:], ps_mg[:])
            nc.vector.tensor_add(r_sb[:], r_sb[:], bsb["b1"])
            nc.vector.tensor_add(r_sb[:], r_sb[:], bsb["be"])
            nc.vector.tensor_add(r_sb[:], r_sb[:], bsb["bg"])
            bso = pp.tile([1, MID], F32)
            nc.vector.tensor_add(bso[:], bsb["bo1"], bsb["bo2"])

            # ---------------- cT[mid, i] = (m1 + r)^T ----------------
            ps_cT = psTp.tile([128, IH], F32, name="ps_cT", tag="pT")
            nc.tensor.matmul(
                ps_cT[:], lhsT=wsb["W1"][:], rhs=noderT[:], start=True, stop=False
            )
            nc.tensor.matmul(
                ps_cT[:], lhsT=r_sb[:], rhs=ones32[:], start=False, stop=True
            )
            cT_sb = pp.tile([128, IH], F32)
            nc.scalar.copy(cT_sb[:], ps_cT[:])

        # ---------------- main streaming loop ----------------
        acc = [None, None]
        for e in range(2):
            acc[e] = accp.tile([128, 4 * MID], F16, name=f"acc{e}")
        first_unit = [True, True]
        copy_ctr = [0]

        if True:
            # Software pipeline: per unit (g, e) emit the transposes and
            # PSUM->SBUF copies (or the group's dma-transpose); the
            # We-matmuls + rank-2 + reduce for a unit are emitted two units
            # later so the PE never head-of-line blocks on the eviction of
            # its own transposes.
            def emit_mm_reduce(st):
                g, e, kind, tsrc = st
                k = _k_of_group(g)
                ps8 = ps8p.tile([128, JG * MID], F32, tag="ps8")
                for half in range(2):
                    if kind == "pe":
                        rhs = tsrc[half][:]
                    else:
                        rhs = (
                            tsrc[:]
                            .rearrange("p (j q c) -> p j q c", q=2, c=128)
                            [:, 4 * half:4 * half + 4, e, :]
                        )
                    nc.tensor.matmul(
                        ps8[:, half * 512:(half + 1) * 512],
                        lhsT=we16[:], rhs=rhs,
                        start=True, stop=False,
                    )
                    for q in range(4):
                        jl = half * 4 + q
                        j = g * JG + jl
                        u = _u_of_j(j)
                        nc.tensor.matmul(
                            ps8[:, jl * MID:(jl + 1) * MID],
                            lhsT=m2r2[32 * k:32 * k + 2,
                                      u * 128:(u + 1) * 128],
                            rhs=adjr2[32 * k:32 * k + 2,
                                      u * 256 + e * 128:u * 256 + e * 128 + 128],
                            start=False, stop=(q == 3),
                            tile_position=(32 * k, 0),
                        )
                # ---- reduce: evict slots 4-7, fold with slots 0-3, accum ----
                ev = evp.tile([128, 512], F32, tag="ev")
                nc.scalar.copy(ev[:], ps8[:, 512:1024])
                if first_unit[e]:
                    nc.vector.tensor_tensor(
                        acc[e][:], ps8[:, 0:512], ev[:], op=mybir.AluOpType.max
                    )
                    first_unit[e] = False
                else:
                    t16 = t16p.tile([128, 512], F16, tag="t16")
                    nc.vector.tensor_tensor(
                        t16[:], ps8[:, 0:512], ev[:], op=mybir.AluOpType.max
                    )
                    nc.vector.tensor_tensor(
                        acc[e][:], acc[e][:], t16[:], op=mybir.AluOpType.max
                    )

            stash = []
            for g in range(repeat * NG):
                g = g % NG
                e_t = ep.tile([128, JG * 2 * D], F16, tag="e")
                nc.gpsimd.dma_start(
                    e_t[:],
                    edge[g * JG:(g + 1) * JG]
                    .rearrange("j (p e) d -> p j e d", p=128),
                )
                if g in DMA_GROUPS:
                    tfb = tfbp.tile([128, JG * 2 * D], F16, tag="tfb")
                    nc.sync.dma_start_transpose(
                        tfb[:].rearrange("p (k c) -> p k c", k=16), e_t[:]
                    )
                    for e in range(2):
                        stash.append((g, e, "dma", tfb))
                        if len(stash) > 2:
                            emit_mm_reduce(stash.pop(0))
                else:
                    for e in range(2):
                        tfs = []
                        for half in range(2):
                            pT = psTp.tile([128, 512], F16, tag="pT")
                            for q in range(4):
                                jl = half * 4 + q
                                nc.tensor.transpose(
                                    pT[:, q * 128:(q + 1) * 128],
                                    e_t[:, (jl * 2 + e) * D:(jl * 2 + e + 1) * D],
                                    ident16[:],
                                )
                            tf = tfp.tile([128, 512], F16, tag="tf")
                            if copy_ctr[0] % DVE_COPY_MOD == 0:
                                nc.vector.tensor_copy(tf[:], pT[:])
                            else:
                                nc.scalar.copy(tf[:], pT[:])
                            copy_ctr[0] += 1
                            tfs.append(tf)
                        stash.append((g, e, "pe", tfs))
                        if len(stash) > 2:
                            emit_mm_reduce(stash.pop(0))
            while stash:
                emit_mm_reduce(stash.pop(0))

            # ---------------- finalize ----------------
            with (
                tc.tile_pool(name="fin_sb", bufs=2) as fsb,
            ):
                fps = psTp
                for e in range(2):
                    f1 = fsb.tile([128, 2 * MID], F16, tag="f1")
                    nc.vector.tensor_tensor(
                        f1[:], acc[e][:, 0:256], acc[e][:, 256:512],
                        op=mybir.AluOpType.max,
                    )
                    mraw = fsb.tile([128, MID], F32, tag="mraw")
                    nc.vector.tensor_tensor(
                        mraw[:], f1[:, 0:128], f1[:, 128:256],
                        op=mybir.AluOpType.max,
                    )
                    # msgs^T [mid, i] = max(mraw + cT, -1e6)
                    msgs = fsb.tile([128, MID], F32, tag="msgs")
                    nc.vector.tensor_add(
                        msgs[:], mraw[:], cT_sb[:, e * MID:(e + 1) * MID]
                    )
                    nc.vector.tensor_scalar_max(msgs[:], msgs[:], -BIG_NUMBER)
                    ps_h = fps.tile([128, OUT], F32, tag="pT")
                    nc.tensor.matmul(
                        ps_h[:], lhsT=msgs[:], rhs=wsb["Wo2"],
                        start=True, stop=False,
                    )
                    nc.tensor.matmul(
                        ps_h[:], lhsT=noderT[:, e * 128:(e + 1) * 128],
                        rhs=wsb["Wo1"], start=False, stop=False,
                    )
                    nc.tensor.matmul(
                        ps_h[:], lhsT=ones32[:, 0:128], rhs=bso[:],
                        start=False, stop=True,
                    )
                    o_sb = fsb.tile([128, OUT], F32, tag="osb")
                    nc.scalar.activation(
                        o_sb[:], ps_h[:], mybir.ActivationFunctionType.Relu
                    )
                    nc.sync.dma_start(out_d[e * 128:(e + 1) * 128, :], o_sb[:])

    nc.finalize()
    _assert_safe_pe_schedule(nc)
    return nc


def _assert_safe_pe_schedule(nc):
    """No two adjacent sub-tile (row-grouped) matmuls with different
    tile_position in the final PE stream (HW crash pattern)."""
    prev = None
    for func in nc.m.functions:
        for block in func.blocks:
            for inst in block.instructions:
                if not isinstance(inst, mybir.InstMatmult):
                    continue
                rows = inst.tile_size[0] if inst.tile_size else 128
                sub = rows < 128
                cur = (sub, tuple(inst.tile_position or (0, 0)))
                if (
                    prev is not None
                    and prev[0] and sub
                    and prev[1] != cur[1]
                ):
                    raise AssertionError(
                        f"unsafe adjacent row-grouped matmuls: {prev} -> {cur}"
                    )
                prev = cur
    return True


_CACHED = {}


def _get_program():
    if "nc" not in _CACHED:
        _CACHED["nc"] = _build_program()
    return _CACHED["nc"]


# receiver parity permutation: column e*128+c of a kernel tensor holds
# original receiver i = 2c+e of the core's half
_PERM = np.concatenate([np.arange(0, IH, 2), np.arange(1, IH, 2)])


def kernel(**inputs) -> np.ndarray:
    nc = _get_program()

    def f32(x):
        return np.ascontiguousarray(np.asarray(x, dtype=np.float32))

    node_fts = f32(inputs["node_fts"])
    edge_fts = f32(inputs["edge_fts"])
    graph_fts = f32(inputs["graph_fts"])
    adj_mat = np.ascontiguousarray(np.asarray(inputs["adj_mat"], dtype=np.int32))

    shared = {}
    shared["wpack"] = np.ascontiguousarray(np.concatenate(
        [f32(inputs[w]) for w in ("W2", "W1", "Wg", "Wo1", "Wo2")], axis=1
    ))
    shared["bpack"] = np.ascontiguousarray(np.concatenate(
        [f32(inputs[b]).reshape(1, MID)
         for b in ("b1", "b2", "be", "bg", "bo1", "bo2")], axis=1
    ))
    shared["We"] = f32(inputs["We"])

    in_maps = []
    for c in range(NCORES):
        b, ih = c // 2, c % 2
        sl = slice(ih * IH, (ih + 1) * IH)
        m = dict(shared)
        m["edge"] = np.ascontiguousarray(edge_fts[b, :, sl, :])
        m["nodeT"] = np.ascontiguousarray(node_fts[b].T)
        m["noderT"] = np.ascontiguousarray(node_fts[b, sl, :][_PERM].T)
        m["graph"] = np.ascontiguousarray(graph_fts[b]).reshape(1, D)
        m["adj"] = np.ascontiguousarray(adj_mat[b, :, sl][:, _PERM])
        in_maps.append(m)

    res = run_bass_kernel_spmd(nc, in_maps, list(range(NCORES)))

    out = np.empty((B, N, OUT), dtype=np.float32)
    for c in range(NCORES):
        b, ih = c // 2, c % 2
        blk = out[b, ih * IH:(ih + 1) * IH, :]
        blk[_PERM] = res.results[c]["out"]
    return out
